# revision 4
# baseline (speedup 1.0000x reference)
"""Trainium2 Bass kernel for nn_NodeEncoder (2-layer SAGEConv GNN).

Self-contained: takes FULL inputs, shards receivers across 8 NeuronCores,
runs a Bass/Tile kernel via run_bass_kernel_spmd, returns the FULL output.

Algorithm per layer (SAGEConv, degree_norm=True, self loops):
  x_upd[r] = dr[r]^-1.5 * sum_{e: recv=r} ds[s_e]^-0.5 * x[s_e]   (incl. self)
  out = concat([x, x_upd]) @ W + b   (+relu after layer 1)

Device mapping:
  - gather x[s] rows (512B) via SWDGE dma_gather from a 4-banked table
  - weighted one-hot (iota == recv_rel)*w built in one DVE tensor_scalar
  - PE matmul lhsT=X_g[e,f], rhs=onehot[e,n] accumulates summed^T [f,n] in PSUM
  - self loop = matmul lhsT=x_win[n,f], rhs=diag(selfw)
  - dense = 2 matmuls with W-halves as lhsT; ACT applies bias(+relu)
  - PE transposes move between row-major and feature-major
  - AllGather shares layer-1 activations across cores for layer-2 gathers
"""

import numpy as np

N = 100000
E = 600000
D = 128
NC = 8
P = 128

SLICE = N // NC            # 12500 nodes per core
NW = (SLICE + P - 1) // P  # 98 windows per core
SLICE_PAD = NW * P         # 12544
NPAD = SLICE_PAD * NC      # 100352 padded rows
NBANKS = 4
BROWS = NPAD // NBANKS     # 25088 rows per bank (< 32768 for int16)
GATHER_BATCH = 2048        # max idxs per dma_gather instruction

_last_results = None       # stashed BassKernelResults for test harness


def _make_layout(caps):
    """Compile-time layout shared by all cores: chunk positions per bank,
    gather batches, pair list."""
    chunk_of = np.zeros((NW, NBANKS), np.int64)
    nchunks_b = np.zeros(NBANKS, np.int64)
    for b in range(NBANKS):
        pos = 0
        for k in range(NW):
            chunk_of[k, b] = pos
            pos += caps[k, b]
        nchunks_b[b] = pos

    batches = []   # (bank, start_chunk, nchunks)
    for b in range(NBANKS):
        c0 = 0
        while c0 < nchunks_b[b]:
            nb = min(GATHER_BATCH // P, int(nchunks_b[b]) - c0)
            batches.append((b, c0, nb))
            c0 += nb

    pairs = []     # (window, bank, chunk_pos) in window order
    maxcap = int(caps.max())
    pair_arr = np.full((NW, NBANKS, maxcap), -1, np.int64)
    for k in range(NW):
        for b in range(NBANKS):
            for j in range(int(caps[k, b])):
                pair_arr[k, b, j] = len(pairs)
                pairs.append((k, b, int(chunk_of[k, b] + j)))
    return chunk_of, nchunks_b, batches, pairs, pair_arr


def _layout_core(edges, chunk_of, nchunks_b, pair_arr, npairs):
    """Vectorized slot assignment for one (core, layer).
    edges: (brow:int16, bank, k, rloc, ds_e, dr_e) sorted by (k, bank)."""
    brow, bank, k, rloc, ds_e, dr_e = edges
    n = len(bank)
    gid = k * NBANKS + bank
    # within-group offset
    change = np.empty(n, bool)
    change[0] = True
    change[1:] = gid[1:] != gid[:-1]
    first = np.where(change)[0]
    grp = np.cumsum(change) - 1
    f = np.arange(n) - first[grp]
    cpos = chunk_of[k, bank] + f // P
    p = f % P
    pi = pair_arr[k, bank, f // P]
    assert (pi >= 0).all()

    idx16 = []
    for b in range(NBANKS):
        m = bank == b
        st = np.zeros(int(nchunks_b[b]) * P, np.int16)
        st[cpos[m] * P + p[m]] = brow[m]
        cols = len(st) // 16
        a = st.reshape(cols, 16).T.copy()
        idx16.append(np.tile(a, (8, 1)))          # replicate for 8 Q7 cores

    recv = np.full((P, npairs), -1000.0, np.float32)
    dse = np.ones((P, npairs), np.float32)
    dre = np.ones((P, npairs), np.float32)
    recv[p, pi] = rloc
    dse[p, pi] = ds_e
    dre[p, pi] = dr_e
    return idx16, recv, dse, dre


def _build_program(caps, chunk_of, nchunks_b, batches, pairs):
    import concourse.bacc as bacc
    import concourse.mybir as mybir
    import concourse.tile as tile
    from concourse.masks import make_identity

    DT = mybir.dt.float32
    npairs = len(pairs)
    nwin = NW
    nc = bacc.Bacc("TRN2", target_bir_lowering=False, num_swdge_queues=4)

    x0 = nc.dram_tensor("x0", [NPAD, D], DT, kind="ExternalInput")
    w1 = nc.dram_tensor("w1", [2 * D, D], DT, kind="ExternalInput")
    b1 = nc.dram_tensor("b1", [D, 1], DT, kind="ExternalInput")
    w2 = nc.dram_tensor("w2", [2 * D, D], DT, kind="ExternalInput")
    b2 = nc.dram_tensor("b2", [D, 1], DT, kind="ExternalInput")
    idxcols = int(nchunks_b.sum()) * P // 16
    gidx1 = nc.dram_tensor("gidx1", [P, idxcols], mybir.dt.int16, kind="ExternalInput")
    gidx2 = nc.dram_tensor("gidx2", [P, idxcols], mybir.dt.int16, kind="ExternalInput")
    recv1 = nc.dram_tensor("recv1", [P, npairs], DT, kind="ExternalInput")
    recv2 = nc.dram_tensor("recv2", [P, npairs], DT, kind="ExternalInput")
    dse1 = nc.dram_tensor("dse1", [P, npairs], DT, kind="ExternalInput")
    dre1 = nc.dram_tensor("dre1", [P, npairs], DT, kind="ExternalInput")
    dse2 = nc.dram_tensor("dse2", [P, npairs], DT, kind="ExternalInput")
    dre2 = nc.dram_tensor("dre2", [P, npairs], DT, kind="ExternalInput")
    dsn = nc.dram_tensor("dsn", [P, nwin], DT, kind="ExternalInput")
    drn = nc.dram_tensor("drn", [P, nwin], DT, kind="ExternalInput")
    smask = nc.dram_tensor("smask", [P, nwin], DT, kind="ExternalInput")
    h1s = nc.dram_tensor("h1s", [SLICE_PAD, D], DT)
    h1f = nc.dram_tensor("h1f", [NPAD, D], DT, addr_space="Shared")
    out = nc.dram_tensor("out", [SLICE_PAD, D], DT, kind="ExternalOutput")

    bank_col0 = np.concatenate([[0], np.cumsum(nchunks_b * P // 16)]).astype(int)
    # per-bank ordered list of batch ids
    bank_batches = {b: [bi for bi, (bb, _, _) in enumerate(batches) if bb == b]
                    for b in range(NBANKS)}
    chunk_to_batch = {}
    for bi, (b, c0, nchk) in enumerate(batches):
        for j in range(nchk):
            chunk_to_batch[(b, c0 + j)] = (bi, j)

    with tile.TileContext(nc) as tc:
        with tc.tile_pool(name="const", bufs=1) as cpool, \
             tc.tile_pool(name="meta", bufs=1) as mpool, \
             tc.tile_pool(name="gat", bufs=2) as gpool, \
             tc.tile_pool(name="win", bufs=3) as wpool, \
             tc.tile_pool(name="oh", bufs=6) as ohpool, \
             tc.tile_pool(name="epi", bufs=3) as epool, \
             tc.tile_pool(name="ps", bufs=2, space="PSUM") as pspool, \
             tc.tile_pool(name="ph", bufs=2, space="PSUM") as phpool, \
             tc.tile_pool(name="pt", bufs=2, space="PSUM") as ptpool, \
             tc.tile_pool(name="po", bufs=2, space="PSUM") as popool:

            ident = cpool.tile([P, P], DT)
            make_identity(nc, ident[:])
            iota_i = cpool.tile([P, P], mybir.dt.int32)
            nc.gpsimd.iota(iota_i[:], pattern=[[1, P]], base=0, channel_multiplier=0)
            iota_f = cpool.tile([P, P], DT)
            nc.vector.tensor_copy(iota_f[:], iota_i[:])
            iop_i = cpool.tile([P, 1], mybir.dt.int32)
            nc.gpsimd.iota(iop_i[:], pattern=[[0, 1]], base=0, channel_multiplier=1)
            iop_f = cpool.tile([P, 1], DT)
            nc.vector.tensor_copy(iop_f[:], iop_i[:])

            wa = [cpool.tile([P, D], DT, tag=f"wa{l}", name=f"wa{l}") for l in range(2)]
            wb = [cpool.tile([P, D], DT, tag=f"wb{l}", name=f"wb{l}") for l in range(2)]
            bias = [cpool.tile([P, 1], DT, tag=f"bias{l}", name=f"bias{l}") for l in range(2)]
            for li, (wt, bt) in enumerate(((w1, b1), (w2, b2))):
                nc.sync.dma_start(out=wa[li][:], in_=wt[0:P, :])
                nc.sync.dma_start(out=wb[li][:], in_=wt[P:2 * P, :])
                nc.sync.dma_start(out=bias[li][:], in_=bt[:, :])

            gidx_sb = [mpool.tile([P, idxcols], mybir.dt.int16, tag=f"gidx{l}", name=f"gidx{l}")
                       for l in range(2)]
            nc.sync.dma_start(out=gidx_sb[0][:], in_=gidx1[:])
            nc.sync.dma_start(out=gidx_sb[1][:], in_=gidx2[:])
            recv_sb = [mpool.tile([P, npairs], DT, tag=f"recv{l}", name=f"recv{l}") for l in range(2)]
            nc.sync.dma_start(out=recv_sb[0][:], in_=recv1[:])
            nc.sync.dma_start(out=recv_sb[1][:], in_=recv2[:])

            # per-edge weight w = (ds * dr^3) ^ -1/2
            wch_sb = []
            for l, (dse_t, dre_t) in enumerate(((dse1, dre1), (dse2, dre2))):
                t_ds = epool.tile([P, npairs], DT, tag="wtmp1")
                t_dr = epool.tile([P, npairs], DT, tag="wtmp2")
                wch = mpool.tile([P, npairs], DT, tag=f"wch{l}")
                nc.sync.dma_start(out=t_ds[:], in_=dse_t[:])
                nc.sync.dma_start(out=t_dr[:], in_=dre_t[:])
                nc.vector.tensor_mul(out=wch[:], in0=t_dr[:], in1=t_dr[:])
                nc.vector.tensor_mul(out=wch[:], in0=wch[:], in1=t_dr[:])
                nc.vector.tensor_mul(out=wch[:], in0=wch[:], in1=t_ds[:])
                nc.vector.reciprocal(out=wch[:], in_=wch[:])
                nc.scalar.sqrt(out=wch[:], in_=wch[:])
                wch_sb.append(wch)

            t_ds = epool.tile([P, nwin], DT, tag="stmp1")
            t_dr = epool.tile([P, nwin], DT, tag="stmp2")
            t_mk = epool.tile([P, nwin], DT, tag="stmp3")
            selfw = mpool.tile([P, nwin], DT)
            nc.sync.dma_start(out=t_ds[:], in_=dsn[:])
            nc.sync.dma_start(out=t_dr[:], in_=drn[:])
            nc.sync.dma_start(out=t_mk[:], in_=smask[:])
            nc.vector.tensor_mul(out=selfw[:], in0=t_dr[:], in1=t_dr[:])
            nc.vector.tensor_mul(out=selfw[:], in0=selfw[:], in1=t_dr[:])
            nc.vector.tensor_mul(out=selfw[:], in0=selfw[:], in1=t_ds[:])
            nc.vector.reciprocal(out=selfw[:], in_=selfw[:])
            nc.scalar.sqrt(out=selfw[:], in_=selfw[:])
            nc.vector.tensor_mul(out=selfw[:], in0=selfw[:], in1=t_mk[:])

            relu_t = mybir.ActivationFunctionType.Relu
            iden_t = mybir.ActivationFunctionType.Identity

            for layer in range(2):
                table = x0 if layer == 0 else h1f
                xsrc = x0 if layer == 0 else h1s
                dst = h1s if layer == 0 else out
                gtiles = {}
                bank_next = [0] * NBANKS      # ordinal into bank_batches[b]

                pi = 0
                for k in range(nwin):
                    xw = wpool.tile([P, D], DT, tag="xw")
                    nc.sync.dma_start(out=xw[:], in_=xsrc[k * P:(k + 1) * P, :])

                    psum = pspool.tile([P, P], mybir.dt.float32, space="PSUM")
                    first = True
                    while pi < len(pairs) and pairs[pi][0] == k:
                        _, b, cpos = pairs[pi]
                        bi, j = chunk_to_batch[(b, cpos)]
                        while bi not in gtiles:
                            nb = bank_batches[b][bank_next[b]]
                            bank_next[b] += 1
                            _, c0, nchk = batches[nb]
                            nidx = nchk * P
                            gt = gpool.tile([P, nchk, D], DT, tag=f"g{b}")
                            col0 = bank_col0[b] + c0 * P // 16
                            nc.gpsimd.dma_gather(
                                gt[:],
                                table[b * BROWS:(b + 1) * BROWS, :],
                                gidx_sb[layer][:, col0:col0 + nidx // 16],
                                nidx, nidx, D,
                                single_packet=False, queue_num=b,
                            )
                            gtiles[nb] = gt
                        gt = gtiles[bi]
                        oh = ohpool.tile([P, P], DT, tag="oh")
                        nc.vector.tensor_scalar(
                            out=oh[:], in0=iota_f[:],
                            scalar1=recv_sb[layer][:, pi:pi + 1],
                            scalar2=wch_sb[layer][:, pi:pi + 1],
                            op0=mybir.AluOpType.is_equal,
                            op1=mybir.AluOpType.mult,
                        )
                        nc.tensor.matmul(
                            out=psum[:], lhsT=gt[:, j, :], rhs=oh[:],
                            start=first, stop=False,
                        )
                        first = False
                        pi += 1

                    dg = ohpool.tile([P, P], DT, tag="dg")
                    nc.vector.tensor_scalar(
                        out=dg[:], in0=iota_f[:],
                        scalar1=iop_f[:, 0:1],
                        scalar2=selfw[:, k:k + 1],
                        op0=mybir.AluOpType.is_equal,
                        op1=mybir.AluOpType.mult,
                    )
                    nc.tensor.matmul(out=psum[:], lhsT=xw[:], rhs=dg[:],
                                     start=first, stop=True)

                    summed = epool.tile([P, P], DT, tag="summed")
                    nc.scalar.copy(out=summed[:], in_=psum[:])
                    pt = ptpool.tile([P, P], mybir.dt.float32, space="PSUM")
                    nc.tensor.transpose(out=pt[:], in_=xw[:], identity=ident[:])
                    xt = epool.tile([P, P], DT, tag="xt")
                    nc.scalar.copy(out=xt[:], in_=pt[:])

                    ph = phpool.tile([P, P], mybir.dt.float32, space="PSUM")
                    nc.tensor.matmul(out=ph[:], lhsT=wa[layer][:], rhs=xt[:],
                                     start=True, stop=False)
                    nc.tensor.matmul(out=ph[:], lhsT=wb[layer][:], rhs=summed[:],
                                     start=False, stop=True)
                    ht = epool.tile([P, P], DT, tag="ht")
                    nc.scalar.activation(
                        out=ht[:], in_=ph[:],
                        func=relu_t if layer == 0 else iden_t,
                        bias=bias[layer][:, 0:1],
                    )
                    po = popool.tile([P, P], mybir.dt.float32, space="PSUM")
                    nc.tensor.transpose(out=po[:], in_=ht[:], identity=ident[:])
                    hrow = epool.tile([P, P], DT, tag="hrow")
                    nc.scalar.copy(out=hrow[:], in_=po[:])
                    nc.sync.dma_start(out=dst[k * P:(k + 1) * P, :], in_=hrow[:])

                if layer == 0:
                    nc.gpsimd.collective_compute(
                        kind="AllGather",
                        op=mybir.AluOpType.bypass,
                        replica_groups=[list(range(NC))],
                        ins=[h1s[:, :]],
                        outs=[h1f[:, :]],
                    )
    nc.compile()
    return nc


def kernel(gid, senders, receivers, is_training, emb_table, W1, b1, W2, b2):
    global _last_results
    from concourse.bass_utils import run_bass_kernel_spmd

    gid = np.asarray(gid)
    s = np.asarray(senders).astype(np.int64)
    r = np.asarray(receivers).astype(np.int64)
    emb = np.asarray(emb_table, dtype=np.float32)
    W1 = np.asarray(W1, np.float32); b1v = np.asarray(b1, np.float32)
    W2 = np.asarray(W2, np.float32); b2v = np.asarray(b2, np.float32)

    x0_full = emb[gid]                      # host indexing (layout only)

    ds = 1 + np.bincount(s, minlength=N)
    dr = 1 + np.bincount(r, minlength=N)
    edge_ds = ds[s].astype(np.float32)
    edge_dr = dr[r].astype(np.float32)

    core_of = r // SLICE
    s_core = s // SLICE
    s_loc = s % SLICE
    s_pad_glob = SLICE_PAD * s_core + s_loc

    # gather per-(core,layer) edge tuples; global capacity map
    per_key = {}
    counts_all = np.zeros((NW, NBANKS), np.int64)
    for c in range(NC):
        m = core_of == c
        r_local = r[m] - c * SLICE
        k = r_local // P
        rloc = (r_local - k * P).astype(np.float32)
        s_rot = SLICE_PAD * ((s_core[m] - c) % NC) + s_loc[m]
        for layer, s_padded in ((0, s_rot), (1, s_pad_glob[m])):
            bank = s_padded // BROWS
            brow = (s_padded % BROWS).astype(np.int16)
            counts = np.zeros((NW, NBANKS), np.int64)
            np.add.at(counts, (k, bank), 1)
            np.maximum(counts_all, counts, out=counts_all)
            order = np.lexsort((bank, k))
            per_key[(c, layer)] = (brow[order], bank[order], k[order],
                                   rloc[order], edge_ds[m][order],
                                   edge_dr[m][order])
    caps = np.maximum((counts_all + P - 1) // P, 1)

    chunk_of, nchunks_b, batches, pairs, pair_arr = _make_layout(caps)
    npairs = len(pairs)

    nc = _build_program(caps, chunk_of, nchunks_b, batches, pairs)

    in_maps = []
    for c in range(NC):
        x0p = np.zeros((NPAD, D), np.float32)
        for rr in range(NC):
            src_c = (c + rr) % NC
            x0p[rr * SLICE_PAD: rr * SLICE_PAD + SLICE] = \
                x0_full[src_c * SLICE:(src_c + 1) * SLICE]
        idx1, recv_1, dse_1, dre_1 = _layout_core(
            per_key[(c, 0)], chunk_of, nchunks_b, pair_arr, npairs)
        idx2, recv_2, dse_2, dre_2 = _layout_core(
            per_key[(c, 1)], chunk_of, nchunks_b, pair_arr, npairs)
        dsn_a = np.ones((P, NW), np.float32)
        drn_a = np.ones((P, NW), np.float32)
        mask_a = np.zeros((P, NW), np.float32)
        loc = np.arange(SLICE)
        kk, pp = loc // P, loc % P
        dsn_a[pp, kk] = ds[c * SLICE + loc]
        drn_a[pp, kk] = dr[c * SLICE + loc]
        mask_a[pp, kk] = 1.0
        in_maps.append({
            "x0": x0p,
            "w1": W1, "b1": b1v.reshape(D, 1),
            "w2": W2, "b2": b2v.reshape(D, 1),
            "gidx1": np.concatenate(idx1, axis=1),
            "gidx2": np.concatenate(idx2, axis=1),
            "recv1": recv_1, "recv2": recv_2,
            "dse1": dse_1, "dre1": dre_1,
            "dse2": dse_2, "dre2": dre_2,
            "dsn": dsn_a, "drn": drn_a, "smask": mask_a,
        })

    res = run_bass_kernel_spmd(nc, in_maps, core_ids=list(range(NC)))
    _last_results = res

    out = np.empty((N, D), np.float32)
    for c in range(NC):
        out[c * SLICE:(c + 1) * SLICE] = res.results[c]["out"][:SLICE]
    return out
